# revision 17
# baseline (speedup 1.0000x reference)
"""Trainium2 Bass kernel for nn_Encoder_62740882260638 (ragged set encoder).

Pure data parallel over 8 NeuronCores + ragged bucketing: sets are sorted
by n on the host, snake-dealt across cores (so every core sees the same n
profile), and each 128-set chunk is processed with only b = ceil4(max n)
token slots instead of 16.  Mean n is 8.5, so bucketing cuts per-token
matmul/vector work to ~62% of the padded version.  The output is
un-permuted on the host.

Structure (v3 — bucketed two-pass software pipeline):
  * Pass A (rank pipeline, issued 3 chunks ahead of pass B): loads x
    token-major, computes rank scores on DVE (exact fp32 accumulate),
    masks padded slots, runs the comparison-count rank via small PE
    matmuls, builds the rank one-hot OH and the n one-hot OHn, and stages
    x to DRAM for the feature-major transpose read.
  * Pass B (per chunk): deepset branch, ed MLP (chunk-PAIRED so matmuls
    run 256 cols), main branch, em MLP (also paired), software-pipelined
    so the in-order PE queue stays busy.
  * Ragged masking is free: padded tokens get rank 16 -> routed to trash
    table rows (0 for the deepset key table; -1e30 for the km rows so
    relu zeroes the hidden).  All tables keep 17 rows regardless of the
    chunk bucket, so only the chunk geometry is parameterized.
"""

from contextlib import ExitStack

import numpy as np

import concourse.bass as bass
import concourse.mybir as mybir
import concourse.tile as tile
from concourse import bacc
from concourse import bass_utils

FP = mybir.dt.float32
FPR = mybir.dt.float32r
BF = mybir.dt.bfloat16
I32 = mybir.dt.int32
AF = mybir.ActivationFunctionType
OP = mybir.AluOpType

B, N, DIM, HID, MAXN1 = 8192, 16, 256, 512, 17
NCORES = 8
SC = B // NCORES  # sets per core (1024)
CS = 128          # sets per chunk
BIG = 1.0e30

# per-bucket chunk geometry: b -> (nsub, tokens per subchunk, sets per sub)
PLAN = {16: (4, 512, 32), 14: (4, 448, 32), 12: (4, 384, 32),
        10: (4, 320, 32), 8: (2, 512, 64), 6: (2, 384, 64),
        4: (1, 512, 128), 2: (1, 256, 128)}
BUCKETS_ALL = (2, 4, 6, 8, 10, 12, 14, 16)


def _ksplit(total):
    return [(a, min(a + 128, total)) for a in range(0, total, 128)]


def build_program(buckets, num_devices=1):
    nc = bacc.Bacc(
        "TRN2", target_bir_lowering=False, debug=False,
        num_devices=num_devices,
    )
    nchunks = len(buckets)
    assert nchunks % 2 == 0
    S = CS * nchunks
    cts = [CS * b for b in buckets]
    offs = np.concatenate([[0], np.cumsum(cts)]).astype(int)
    tot_tok = int(offs[-1])

    def din(name, shape, dtype=FP):
        return nc.dram_tensor(name, shape, dtype, kind="ExternalInput").ap()

    x_d = din("x", [tot_tok, DIM])
    n_d = din("n_i", [S], I32)

    vdW1 = din("vdW1", [DIM, DIM], BF)
    b1d = din("b1d", [DIM, 1])
    vdW2 = din("vdW2", [DIM, DIM], FPR)
    b2d = din("b2d", [DIM, 1])
    vmW1x = din("vmW1x", [DIM, HID], BF)
    b1v = din("b1v", [HID, 1])
    vmW2 = din("vmW2", [HID, HID], BF)
    b2v = din("b2v", [HID, 1])
    Kds = din("Kds", [128, DIM], FPR)
    KRt = din("KRt", [128, 394], FPR)
    kmW2e = din("kmW2e", [394, HID], BF)
    edW1 = din("edW1", [DIM, DIM], FPR)
    edb1 = din("edb1", [DIM, 1])
    edW2 = din("edW2", [DIM, DIM], FPR)
    Wze = din("Wze", [DIM + 1, 906], FPR)
    emW1y = din("emW1y", [HID, 522], FPR)
    EMt = din("EMt", [MAXN1, 522], FPR)
    emW2e = din("emW2e", [522, HID], FPR)
    rankWb = din("rankWb", [128, DIM])
    ident = din("ident", [128, 128], FPR)
    I32rb = {b: din(f"I32r_{b}", [128, PLAN[b][1]], FPR)
             for b in BUCKETS_ALL}
    iota113 = din("iota113", [128, 1])
    iota17 = din("iota17", [MAXN1, 1])
    S8b = {b: din(f"S8_{b}", [128, 8], FPR) for b in BUCKETS_ALL}
    iota8_0 = din("iota8_0", [8, 1])
    iota8_1 = din("iota8_1", [8, 1])
    LTb = {(b, h): din(f"LT_{b}_{h}", [128, 1])
           for b in BUCKETS_ALL for h in range((b + 7) // 8)}
    onesr = din("onesr", [1, 2 * CS], FPR)
    identF = din("identF", [128, 128])

    z_d = nc.dram_tensor("z_out", [S, HID], FP, kind="ExternalOutput").ap()

    with tile.TileContext(nc) as tc, ExitStack() as ctx, \
            nc.allow_low_precision(reason="f32r stores fp32 bits"):
        wpool = ctx.enter_context(tc.tile_pool(name="wpool", bufs=1))
        rk8 = ctx.enter_context(tc.tile_pool(name="rk8", bufs=1))
        glob = ctx.enter_context(tc.tile_pool(name="glob", bufs=2))
        work = ctx.enter_context(tc.tile_pool(name="work", bufs=2))
        work3 = ctx.enter_context(tc.tile_pool(name="work3", bufs=2))
        ps = ctx.enter_context(tc.tile_pool(name="ps", bufs=1, space="PSUM"))
        dstg = ctx.enter_context(
            tc.tile_pool(name="dstg", bufs=4, space="DRAM"))
        xstg = ctx.enter_context(
            tc.tile_pool(name="xstg", bufs=1, space="DRAM"))

        def wload(ap, name):
            """Load a [K, M] weight as a list of <=128-partition K-slabs."""
            k = ap.shape[0]
            if len(ap.shape) == 1 or k <= 128:
                t = wpool.tile(ap.shape, ap.dtype, name=f"w_{name}")
                nc.sync.dma_start(out=t, in_=ap)
                return t
            slabs = []
            for si, (k0, k1) in enumerate(_ksplit(k)):
                t = wpool.tile([k1 - k0] + list(ap.shape[1:]), ap.dtype,
                               name=f"w_{name}_{si}")
                nc.sync.dma_start(out=t, in_=ap[k0:k1])
                slabs.append(t)
            return slabs

        s_vdW1 = wload(vdW1, "vdW1")
        s_b1d = wload(b1d, "b1d")
        s_vdW2 = wload(vdW2, "vdW2")
        s_b2d = wload(b2d, "b2d")
        s_vmW1x = wload(vmW1x, "vmW1x")
        s_b1v = wload(b1v, "b1v")
        s_vmW2 = wload(vmW2, "vmW2")
        s_b2v = wload(b2v, "b2v")
        s_Kds = wload(Kds, "Kds")
        s_KRt = wload(KRt, "KRt")
        s_kmW2e = wload(kmW2e, "kmW2e")
        s_edW1 = wload(edW1, "edW1")
        s_edb1 = wload(edb1, "edb1")
        s_edW2 = wload(edW2, "edW2")
        s_Wze = wload(Wze, "Wze")
        s_emW1y = wload(emW1y, "emW1y")
        s_EMt = wload(EMt, "EMt")
        s_emW2e = wload(emW2e, "emW2e")
        s_rankWb = wload(rankWb, "rankWb")
        s_ident = wload(ident, "ident")
        s_iota113 = wload(iota113, "iota113")
        s_iota17 = wload(iota17, "iota17")
        s_S8 = {b: wload(S8b[b], f"S8_{b}") for b in BUCKETS_ALL}
        s_iota8 = [wload(iota8_0, "iota8_0"), wload(iota8_1, "iota8_1")]
        s_LT = {k: wload(v, f"LT_{k[0]}_{k[1]}") for k, v in LTb.items()}
        s_ones = wload(onesr, "onesr")
        s_identF = wload(identF, "identF")

        def psum(name, shape=(128, 512), tag="mmA", bufs=3, dtype=FP):
            return ps.tile(list(shape), dtype, name=name, tag=tag, bufs=bufs)

        def acopy(out, in_):
            nc.scalar.activation(out, in_, AF.Copy)

        def mm_acc(pt, slabs, msl, rhs_parts, extra=None,
                   keep_open=False):
            """pt += W[:, msl].T @ rhs for one <=128-wide M slice `msl`."""
            if not isinstance(slabs, list):
                slabs = [slabs]
            assert len(slabs) == len(rhs_parts)
            nk = len(rhs_parts) + (1 if extra is not None else 0)
            if keep_open:
                nk = len(rhs_parts) + 1
            for i, (sl, rp) in enumerate(zip(slabs, rhs_parts)):
                assert sl.shape[0] == rp.shape[0], (sl.shape, rp.shape)
                nc.tensor.matmul(
                    pt, sl[:, msl], rp,
                    start=(i == 0), stop=(i == nk - 1))
            if extra is not None:
                l2, r2 = extra
                nc.tensor.matmul(
                    pt, l2, r2,
                    start=False, stop=True)

        # token-major chunk views: x_r[c][p, A, d] = x[off_c + A*128 + p, d]
        x_r = [x_d[int(offs[c]):int(offs[c + 1])]
               .rearrange("(a p) d -> p a d", p=128) for c in range(nchunks)]

        # ---- persistent per-chunk rank products ----
        OHs = [rk8.tile([128, 512], FPR, name=f"OH{c}")
               for c in range(nchunks)]
        OHn_all = rk8.tile([MAXN1, CS * nchunks], FPR, name="OHn_all")
        # DRAM staging for token-major bf16 x (written pass A, read pass B
        # via DMA XBAR transpose)
        xT = [[xstg.tile([cts[c], 128], BF, name=f"xT_{c}_{h}")
               for h in range(2)] for c in range(nchunks)]

        # ---- cross-step state (python handles to live tiles) ----
        st = {}

        def xtm_load(c):
            b = buckets[c]
            hb = b // 2
            ts = []
            for hf in range(2):
                t = work3.tile([128, 8, DIM], FP, name="x_tm")
                nc.sync.dma_start(
                    out=t[:, 0:hb], in_=x_r[c][:, hb * hf:hb * (hf + 1)])
                ts.append(t)
            st[("xtm", c)] = ts

        def xfm_load(c):
            ct = cts[c]
            fm = [work.tile([128, 2048], BF, name=f"x_fm{h}", bufs=3)
                  for h in (0, 1)]
            for h in range(2):
                nc.scalar.dma_start(out=fm[h][:, 0:ct], in_=xT[c][h],
                                    transpose=True)
            st[("xfm", c)] = fm

        def rankA(c):
            """Rank scores + mask + magd write; x transposes -> DRAM."""
            b = buckets[c]
            hb = b // 2
            ct = cts[c]
            x_tm = st.pop(("xtm", c))
            mag16 = work3.tile([128, 16], FP, name="mag16")
            junk = work3.tile([128, DIM], FP, name="junk", bufs=1)
            for A in range(b):
                nc.vector.scalar_tensor_tensor(
                    out=junk, in0=x_tm[A // hb][:, A % hb, :], scalar=0.0,
                    in1=s_rankWb, op0=OP.bypass, op1=OP.mult,
                    accum_out=mag16[:, A:A + 1])
            magd = dstg.tile([2048], FP, name="magd")
            nc.gpsimd.dma_start(
                out=magd[0:ct].rearrange("(a p) -> p a", p=128),
                in_=mag16[:, 0:b])
            st[("magd", c)] = magd
            if c < 3:
                # warmup fast path: PE transposes straight into x_fm (the PE
                # is idle here and this skips the DRAM roundtrip latency)
                fm = [work.tile([128, 2048], BF, name=f"x_fm{h}", bufs=3)
                      for h in (0, 1)]
                for g in range((b + 3) // 4):
                    na = min(4, b - 4 * g)
                    for h in range(2):
                        pt = psum("pt", (128, 4, 128), tag="tr", bufs=2,
                                  dtype=FP)
                        for a in range(na):
                            A = 4 * g + a
                            nc.tensor.transpose(
                                pt[:, a, :],
                                x_tm[A // hb][:, A % hb,
                                              128 * h:128 * (h + 1)],
                                s_identF)
                        dst = fm[h][:, 512 * g:512 * g + 128 * na]                             .rearrange("p (a q) -> p a q", a=na)
                        if h == 0:
                            acopy(dst, pt[:, 0:na, :])
                        else:
                            nc.vector.tensor_copy(out=dst, in_=pt[:, 0:na, :])
                st[("xfm", c)] = fm
                return
            # cast to bf16 and stage token-major halves; pass B transposes
            # via the DMA XBAR (no PE involvement).
            xbf = work3.tile([128, 16, DIM], BF, name="xbf", bufs=1)
            acopy(xbf[:, 0:hb], x_tm[0][:, 0:hb])
            nc.vector.tensor_copy(out=xbf[:, hb:b], in_=x_tm[1][:, 0:hb])
            for h in range(2):
                nc.gpsimd.dma_start(
                    out=xT[c][h].rearrange("(a p) f -> p a f", p=128),
                    in_=xbf[:, 0:b, 128 * h:128 * (h + 1)])

        def rankB1(c):
            """Rank stage 1: mask padded slots, restage mag i-major."""
            b = buckets[c]
            ct = cts[c]
            s0 = c * CS
            magd = st.pop(("magd", c))
            magdv = magd[0:ct].rearrange("(s i) -> i s", i=b)
            n_i32 = glob.tile([128, CS], I32, name="n_i32", bufs=3)
            nc.sync.dma_start(
                out=n_i32,
                in_=n_d[s0:s0 + CS].unsqueeze(0).broadcast_to([128, CS]))
            n_repf = glob.tile([128, CS], FP, name="n_repf", bufs=3)
            nc.vector.tensor_copy(out=n_repf, in_=n_i32)
            # mask padded slots to +BIG in [i, s] layout, restage i-major
            mag_fm = glob.tile([16, CS], FP, name="mag_fm", bufs=3)
            nc.gpsimd.dma_start(out=mag_fm[0:b], in_=magdv)
            inv = glob.tile([16, CS], FP, name="inv", bufs=3)
            nc.vector.tensor_scalar(
                out=inv[0:b], in0=n_repf[0:b], scalar1=s_iota17[0:b],
                scalar2=None, op0=OP.is_le)
            mag_m = glob.tile([16, CS], FP, name="mag_m", bufs=3)
            nc.vector.scalar_tensor_tensor(
                out=mag_m[0:b], in0=inv[0:b], scalar=BIG, in1=mag_fm[0:b],
                op0=OP.mult, op1=OP.add)
            magd2 = dstg.tile([16 * CS], FP, name="magd2")
            nc.gpsimd.dma_start(out=magd2[0:b * CS], in_=mag_m[0:b])
            st[("magd2", c)] = magd2
            st[("n_repf", c)] = n_repf

        def rankB2(c):
            """Rank stage 2: comparison-count ranks, OH / OHn one-hots."""
            b = buckets[c]
            ct = cts[c]
            nsub, nst, sps = PLAN[b]
            magd2 = st.pop(("magd2", c))
            n_repf = st.pop(("n_repf", c))
            magd2v = magd2[0:b * CS].rearrange("(i s) -> i s", s=CS)
            X2 = glob.tile([128, CS], FP, name="X2", bufs=3)
            nc.sync.dma_start(
                out=X2[0:8 * b],
                in_=magd2v.unsqueeze(0).broadcast_to([8, b, CS]))
            rankd = dstg.tile([2048], FPR, name="rankd")
            rankdT = rankd[0:ct].rearrange("(s i) -> i s", i=b)
            for h in range((b + 7) // 8):
                jw = min(8, b - 8 * h)
                pw = jw * b
                X1 = glob.tile([128, CS], FP, name="X1", bufs=3)
                nc.sync.dma_start(
                    out=X1[0:pw],
                    in_=magd2v[8 * h:8 * h + jw, :].unsqueeze(1)
                    .broadcast_to([jw, b, CS]))
                cmp = glob.tile([128, CS], FPR, name="cmp", bufs=3)
                eq = glob.tile([128, CS], FP, name="eq", bufs=3)
                nc.vector.tensor_tensor(
                    out=cmp[0:pw], in0=X2[0:pw], in1=X1[0:pw], op=OP.is_lt)
                nc.vector.tensor_tensor(
                    out=eq[0:pw], in0=X2[0:pw], in1=X1[0:pw],
                    op=OP.is_equal)
                nc.vector.scalar_tensor_tensor(
                    out=cmp[0:pw], in0=eq[0:pw], scalar=s_LT[(b, h)][0:pw],
                    in1=cmp[0:pw], op0=OP.mult, op1=OP.add)
                pr = psum("pr", (8, CS), tag="sm", bufs=1)
                nc.tensor.matmul(pr[0:jw], s_S8[b][0:pw, 0:jw], cmp[0:pw])
                rh = glob.tile([8, CS], FP, name=f"rh{h}", bufs=2)
                nc.vector.tensor_copy(out=rh[0:jw], in_=pr[0:jw])
                # rank_m = rank + inv * (16 - rank), per 8-row half
                ih = glob.tile([8, CS], FP, name=f"ih{h}", bufs=2)
                nc.vector.tensor_scalar(
                    out=ih[0:jw], in0=n_repf[0:jw],
                    scalar1=s_iota8[h][0:jw],
                    scalar2=None, op0=OP.is_le)
                th = glob.tile([8, CS], FPR, name=f"th{h}", bufs=2)
                nc.vector.tensor_scalar(
                    out=th[0:jw], in0=rh[0:jw], scalar1=-1.0, scalar2=16.0,
                    op0=OP.mult, op1=OP.add)
                nc.vector.tensor_tensor(
                    out=th[0:jw], in0=th[0:jw].bitcast(FP), in1=ih[0:jw],
                    op=OP.mult)
                nc.vector.tensor_tensor(
                    out=th[0:jw], in0=th[0:jw].bitcast(FP), in1=rh[0:jw],
                    op=OP.add)
                nc.sync.dma_start(out=rankdT[8 * h:8 * h + jw, :],
                                  in_=th[0:jw])
            # one-hot tile: OH[32g + r, tok_of_subchunk_g] = (rank == r)
            OH = OHs[c]
            OHf = glob.tile([128, 512], FP, name="OHf", bufs=2)
            for g in range(nsub):
                nc.sync.dma_start(
                    out=OHf[32 * g:32 * (g + 1), 0:nst].bitcast(FPR),
                    in_=rankd[nst * g:nst * (g + 1)].unsqueeze(0)
                    .broadcast_to([32, nst]))
            nc.vector.tensor_scalar(
                out=OH[0:32 * nsub, 0:nst], in0=OHf[0:32 * nsub, 0:nst],
                scalar1=s_iota113[0:32 * nsub], scalar2=None,
                op0=OP.is_equal)
            # n one-hot for the em MLP
            nc.vector.tensor_scalar(
                out=OHn_all[:, CS * c:CS * (c + 1)], in0=n_repf[0:MAXN1],
                scalar1=s_iota17, scalar2=None, op0=OP.is_equal)

        def rankB(c):
            rankB1(c)
            rankB2(c)

        def deepset(c):
            b = buckets[c]
            nsub, nst, sps = PLAN[b]
            # stage the zpT-routing table for mainphase(c) (runs next iter)
            i32c = glob.tile([128, 512], FPR, name="i32c", bufs=2)
            nc.sync.dma_start(out=i32c[:, 0:nst], in_=I32rb[b][:, 0:nst])
            st[("i32", c)] = i32c
            x_fm = st[("xfm", c)]
            half = CS * (c % 2)
            if c % 2 == 0:
                st[("y2ds", c // 2)] = [
                    glob.tile([128, 2 * CS], FPR, name=f"y2ds{m}")
                    for m in (0, 1)]
            y2ds = st[("y2ds", c // 2)]
            OH = OHs[c]
            for ns in range(nsub):
                tsl = slice(nst * ns, nst * (ns + 1))
                xp = [x_fm[0][:, tsl], x_fm[1][:, tsl]]
                oh = OH[32 * ns:32 * ns + MAXN1, 0:nst]
                Hd = []
                for m in range(2):
                    pd = psum(f"pd{m}")
                    mm_acc(pd[:, 0:nst], s_vdW1,
                           slice(128 * m, 128 * (m + 1)), xp)
                    hd = work3.tile([128, 512], FPR, name=f"Hd{m}", bufs=1)
                    nc.scalar.activation(hd[:, 0:nst], pd[:, 0:nst],
                                         AF.Relu, bias=s_b1d[m])
                    Hd.append(hd[:, 0:nst])
                for m in range(2):
                    pg = psum(f"pg{m}", tag="mmB", bufs=2)
                    nc.tensor.matmul(
                        pg[:, 0:nst],
                        s_Kds[32 * ns:32 * ns + MAXN1,
                              128 * m:128 * (m + 1)],
                        oh, tile_position=(32 * ns, 0))
                    kg = work3.tile([128, 512], FP, name="KG", bufs=1)
                    nc.vector.tensor_copy(out=kg[:, 0:nst], in_=pg[:, 0:nst])
                    pv = psum(f"pv{m}")
                    mm_acc(pv[:, 0:nst], s_vdW2,
                           slice(128 * m, 128 * (m + 1)), Hd)
                    pds = psum(f"Pds{m}", tag="mmB", bufs=2)
                    nc.vector.scalar_tensor_tensor(
                        out=pds[:, 0:nst], in0=pv[:, 0:nst], scalar=s_b2d[m],
                        in1=kg[:, 0:nst], op0=OP.add, op1=OP.mult)
                    nc.vector.tensor_reduce(
                        out=y2ds[m][:, half + sps * ns:half + sps * (ns + 1)],
                        in_=pds[:, 0:nst].rearrange("p (s i) -> p s i", i=b),
                        axis=mybir.AxisListType.X, op=OP.add)

        def edpair(k):
            """ed MLP + z projections for chunk pair (2k, 2k+1)."""
            y2ds = st.pop(("y2ds", k))
            y2ds = [t[:, :] for t in y2ds]
            He = []
            for m in range(2):
                pe = psum(f"pe{m}", (128, 2 * CS))
                mm_acc(pe, s_edW1, slice(128 * m, 128 * (m + 1)), y2ds)
                he = glob.tile([128, 2 * CS], FPR, name=f"He{m}", bufs=1)
                nc.scalar.activation(he, pe, AF.Relu, bias=s_edb1[m])
                He.append(he)
            ze = []
            for m in range(2):
                pz = psum(f"pz{m}", (128, 2 * CS), tag="mmB", bufs=2)
                mm_acc(pz, s_edW2, slice(128 * m, 128 * (m + 1)), He)
                z1 = glob.tile([128, 2 * CS], FPR, name=f"ze{m}", bufs=1)
                acopy(z1, pz)
                ze.append(z1)
            for cp in range(2):
                c = 2 * k + cp
                csl2 = slice(CS * cp, CS * (cp + 1))
                zpT_s = glob.tile([128, 906], FPR, name="zpT_s")
                for half, csl in ((0, slice(0, 452)), (1, slice(452, 906))):
                    w = csl.stop - csl.start
                    pzt = psum("pzt", (128, 454), tag="mmB", bufs=2)
                    mm_acc(pzt[:, :w], [ze[0][:, csl2], ze[1][:, csl2],
                                        s_ones[:, 0:CS]], slice(None),
                           [sw[:, csl] for sw in s_Wze])
                    acopy(zpT_s[:, csl], pzt[:, :w])
                st[("zpT", c)] = zpT_s

        def mainphase(c):
            b = buckets[c]
            nsub, nst, sps = PLAN[b]
            x_fm = st.pop(("xfm", c))
            zpT_s = st.pop(("zpT", c))
            i32c = st.pop(("i32", c))
            half = CS * (c % 2)
            if c % 2 == 0:
                st[("y2m", c // 2)] = [
                    glob.tile([128, 2 * CS], FPR, name=f"y2m{m}")
                    for m in range(4)]
            y2m = st[("y2m", c // 2)]
            OH = OHs[c]
            for ns in range(nsub):
                tsl = slice(nst * ns, nst * (ns + 1))
                xp = [x_fm[0][:, tsl], x_fm[1][:, tsl]]
                oh = OH[32 * ns:32 * ns + MAXN1, 0:nst]
                i32 = i32c[sps * ns:sps * (ns + 1), 0:nst]
                zsl = slice(sps * ns, sps * (ns + 1))
                tp = (sps * ns, 0)
                Hv = []
                for m in range(4):
                    pvm = psum(f"pvm{m}")
                    mm_acc(pvm[:, 0:nst], s_vmW1x,
                           slice(128 * m, 128 * (m + 1)), xp,
                           keep_open=True)
                    nc.tensor.matmul(
                        pvm[:, 0:nst], zpT_s[zsl, 128 * m:128 * (m + 1)],
                        i32, start=False, stop=True,
                        tile_position=tp)
                    hv = work3.tile([128, 512], BF, name=f"Hv{m}", bufs=1)
                    nc.scalar.activation(hv[:, 0:nst], pvm[:, 0:nst],
                                         AF.Relu, bias=s_b1v[m])
                    Hv.append(hv[:, 0:nst])
                Hk = []
                for m in range(4):
                    mw = 128 if m < 3 else 10
                    pkm = psum(f"pkm{m}")
                    nc.tensor.matmul(
                        pkm[0:mw, 0:nst],
                        s_KRt[32 * ns:32 * ns + MAXN1,
                              128 * m:128 * m + mw],
                        oh, start=True, stop=False,
                        tile_position=(32 * ns, 0))
                    nc.tensor.matmul(
                        pkm[0:mw, 0:nst],
                        zpT_s[zsl, 512 + 128 * m:512 + 128 * m + mw],
                        i32, start=False, stop=True,
                        tile_position=tp)
                    hk = work3.tile([mw, 512], BF, name=f"Hk{m}", bufs=1)
                    nc.scalar.activation(hk[:, 0:nst], pkm[0:mw, 0:nst],
                                         AF.Relu)
                    Hk.append(hk[:, 0:nst])
                for m in range(4):
                    pK = psum(f"pK{m}", tag="mmB", bufs=2)
                    mm_acc(pK[:, 0:nst], s_kmW2e,
                           slice(128 * m, 128 * (m + 1)), Hk)
                    km = work3.tile([128, 512], FP, name="Km", bufs=1)
                    acopy(km[:, 0:nst], pK[:, 0:nst])
                    pV = psum(f"pV{m}")
                    mm_acc(pV[:, 0:nst], s_vmW2,
                           slice(128 * m, 128 * (m + 1)), Hv)
                    pmt = psum(f"Pm{m}", tag="mmB", bufs=2)
                    nc.vector.scalar_tensor_tensor(
                        out=pmt[:, 0:nst], in0=pV[:, 0:nst], scalar=s_b2v[m],
                        in1=km[:, 0:nst], op0=OP.add, op1=OP.mult)
                    nc.vector.tensor_reduce(
                        out=y2m[m][:, half + sps * ns:half + sps * (ns + 1)],
                        in_=pmt[:, 0:nst].rearrange("p (s i) -> p s i", i=b),
                        axis=mybir.AxisListType.X, op=OP.add)

        def empair(k):
            """em MLP + output for chunk pair (2k, 2k+1)."""
            y2m = st.pop(("y2m", k))
            ohn = OHn_all[:, 2 * CS * k:2 * CS * (k + 1)]
            Hm = []
            for m in range(5):
                mw = 128 if m < 4 else 10
                pem = psum(f"pem{m}", (128, 2 * CS))
                pem_v = pem[:mw, :] if mw != 128 else pem
                mm_acc(pem_v, s_emW1y, slice(128 * m, 128 * m + mw), y2m,
                       extra=(s_EMt[:, 128 * m:128 * m + mw], ohn))
                hm = glob.tile([mw, 2 * CS], FPR, name=f"Hm{m}", bufs=1)
                nc.scalar.activation(hm, pem_v, AF.Relu)
                Hm.append(hm)
            zo = []
            for m in range(4):
                pzo = psum(f"pzo{m}", (128, 2 * CS), tag="mmB", bufs=2)
                mm_acc(pzo, s_emW2e, slice(128 * m, 128 * (m + 1)), Hm)
                z1 = glob.tile([128, 2 * CS], FPR, name=f"zo{m}", bufs=1)
                acopy(z1, pzo)
                zo.append(z1)
            for cp in range(2):
                s0 = (2 * k + cp) * CS
                zt = psum("zt", (128, 4, 128), tag="tr", bufs=2, dtype=FPR)
                for m in range(4):
                    nc.tensor.transpose(
                        zt[:, m, :], zo[m][:, CS * cp:CS * (cp + 1)],
                        s_ident)
                z_tm = glob.tile([128, 4, 128], FP, name="z_tm", bufs=1)
                nc.vector.tensor_copy(out=z_tm, in_=zt.bitcast(FP))
                nc.scalar.dma_start(
                    out=z_d[s0:s0 + CS, :].rearrange("s (m f) -> s m f", m=4),
                    in_=z_tm)

        # ---------------- schedule ----------------
        # prologue: pipeline the three initial rank chains (stage-split so
        # chunk c+1's DMAs fly while chunk c's compute waits)
        xtm_load(0)
        rankA(0)
        xtm_load(1)
        rankA(1)
        rankB1(0)
        xtm_load(2)
        rankA(2)
        rankB1(1)
        rankB2(0)
        rankB1(2)
        rankB2(1)
        rankB2(2)
        xtm_load(3)
        for c in range(nchunks):
            deepset(c)
            if c + 3 < nchunks:
                rankA(c + 3)
            if c % 2 == 1:
                edpair((c - 1) // 2)
            if c >= 1:
                mainphase(c - 1)
            if c % 2 == 1 and c >= 3:
                empair((c - 3) // 2)
            if c + 3 < nchunks:
                rankB(c + 3)
            if c + 4 < nchunks:
                xtm_load(c + 4)
            if 3 <= c + 2 < nchunks:
                # issued after mainphase(c-1): bufs=3 WAR rotation on x_fm
                # is against already-issued readers
                xfm_load(c + 2)
        mainphase(nchunks - 1)
        empair(nchunks // 2 - 1)

    nc.compile()
    return nc


def make_tables(inp):
    """Host-side weight preprocessing -> dict of extra input arrays."""
    f = np.float32
    keys = ("rank_W", "kd_W1", "kd_b1", "kd_W2", "kd_b2", "vd_W1", "vd_b1",
            "vd_W2", "vd_b2", "ed_W1", "ed_b1", "ed_W2", "ed_b2", "km_W1",
            "km_b1", "km_W2", "km_b2", "vm_W1", "vm_b1", "vm_W2", "vm_b2",
            "em_W1", "em_b1", "em_W2", "em_b2")
    g = {k: np.asarray(inp[k], f) for k in keys}

    def A(v):
        return np.ascontiguousarray(v, dtype=f)

    import ml_dtypes

    def Bc(v):
        return np.ascontiguousarray(np.asarray(v, f).astype(ml_dtypes.bfloat16))

    kd_h = np.maximum(g["kd_W1"][:16] + g["kd_b1"][None, :], 0.0)
    Kds16 = kd_h @ g["kd_W2"] + g["kd_b2"][None, :]
    Kds17 = np.vstack([Kds16, np.zeros((1, DIM), f)])
    # km first-layer position rows; extra col 392 = valid-mask generator
    # (one-hot rhs sums to 1 -> relu(col392 row) == mask row)
    KRt17 = np.vstack([g["km_W1"][:16] + g["km_b1"][None, :],
                       np.full((1, 392), -BIG, f)])
    KRt17 = np.hstack([KRt17,
                       np.concatenate([np.ones((16, 1), f),
                                       np.zeros((1, 1), f)]),
                       np.zeros((MAXN1, 1), f)])

    def rep4(tab):
        out = np.zeros((128, tab.shape[1]), f)
        for gi in range(4):
            out[32 * gi:32 * gi + MAXN1] = tab
        return out

    Kds = rep4(Kds17)
    KRt = rep4(KRt17)
    kmW2e = np.vstack([g["km_W2"], g["km_b2"][None, :],
                       np.zeros((1, HID), f)])

    Wz = np.hstack([g["vm_W1"][DIM:2 * DIM], g["km_W1"][MAXN1:MAXN1 + DIM],
                    np.zeros((DIM, 2), f)])
    Wze = np.vstack([Wz, (g["ed_b2"] @ Wz)[None, :]])

    # em first layer; extra col 520 = ones generator via the n one-hot
    EMt = np.hstack([g["em_W1"][HID:HID + MAXN1] + g["em_b1"][None, :],
                     np.ones((MAXN1, 1), f), np.zeros((MAXN1, 1), f)])
    emW1y = np.hstack([g["em_W1"][:HID], np.zeros((HID, 2), f)])
    emW2e = np.vstack([g["em_W2"], g["em_b2"][None, :],
                       np.zeros((1, HID), f)])

    p = np.arange(128)
    iota113 = np.where(p % 32 < MAXN1, p % 32, 99).astype(f)[:, None]

    tabs = {
        "vdW1": Bc(g["vd_W1"]), "b1d": A(g["vd_b1"][:, None]),
        "vdW2": A(g["vd_W2"]), "b2d": A(g["vd_b2"][:, None]),
        "vmW1x": Bc(g["vm_W1"][:DIM]), "b1v": A(g["vm_b1"][:, None]),
        "vmW2": Bc(g["vm_W2"]), "b2v": A(g["vm_b2"][:, None]),
        "Kds": A(Kds), "KRt": A(KRt), "kmW2e": Bc(kmW2e),
        "edW1": A(g["ed_W1"]), "edb1": A(g["ed_b1"][:, None]),
        "edW2": A(g["ed_W2"]), "Wze": A(Wze),
        "emW1y": A(emW1y), "EMt": A(EMt), "emW2e": A(emW2e),
        "rankWb": A(np.tile(g["rank_W"].T, (128, 1))),
        "ident": A(np.eye(128)),
        "iota113": A(iota113),
        "iota17": A(np.arange(MAXN1)[:, None]),
        "onesr": A(np.ones((1, 2 * CS))),
        "identF": A(np.eye(128)),
        "iota8_0": A(np.arange(8)[:, None]),
        "iota8_1": A(np.arange(8, 16)[:, None]),
    }
    # per-bucket segment / rank tables
    for b in BUCKETS_ALL:
        nsub, nst, sps = PLAN[b]
        t = np.arange(nst)
        seg = (t[None, :] // b == np.arange(sps)[:, None]).astype(f)
        I32r = np.zeros((128, nst), f)
        for gi in range(nsub):
            I32r[sps * gi:sps * (gi + 1)] = seg
        tabs[f"I32r_{b}"] = A(I32r)
        S8 = np.zeros((128, 8), f)
        pv = np.arange(8 * b)
        S8[pv, pv // b] = 1.0
        tabs[f"S8_{b}"] = A(S8)
        for h in range((b + 7) // 8):
            lt = np.zeros((128, 1), f)
            lt[pv, 0] = ((pv % b) < (pv // b + 8 * h)).astype(f)
            tabs[f"LT_{b}_{h}"] = A(lt)
    return tabs


def _ceil4(v):
    return int((int(v) + 1) // 2) * 2


_prog_cache = {}


def _get_program(buckets, num_devices):
    key = (tuple(buckets), num_devices)
    if key not in _prog_cache:
        _prog_cache[key] = build_program(tuple(buckets), num_devices)
    return _prog_cache[key]


def plan_shard(n):
    """Host-side set->core assignment and per-chunk bucket plan.

    Returns (cores_sets [NCORES][SC], buckets tuple[nchunks])."""
    order = np.argsort(n, kind="stable")[::-1]  # descending n
    cores_sets = [[] for _ in range(NCORES)]
    for k in range(len(order)):
        r, q = divmod(k, NCORES)
        c = q if r % 2 == 0 else NCORES - 1 - q
        cores_sets[c].append(int(order[k]))
    nchunks = SC // CS
    buckets = []
    for j in range(nchunks):
        mx = max(int(np.max(n[cs[CS * j:CS * (j + 1)]]))
                 for cs in cores_sets)
        buckets.append(_ceil4(mx))
    return [np.asarray(cs) for cs in cores_sets], tuple(buckets)


def prepare(inputs):
    """Build (program, per-core input maps, set assignment) for inputs."""
    x = np.ascontiguousarray(np.asarray(inputs["x"], np.float32))
    n = np.ascontiguousarray(np.asarray(inputs["n"], np.int32))
    cores_sets, buckets = plan_shard(n)
    nc = _get_program(buckets, NCORES)
    tabs = make_tables(inputs)
    in_maps = []
    for c in range(NCORES):
        cs = cores_sets[c]
        parts = []
        for j, b in enumerate(buckets):
            sel = cs[CS * j:CS * (j + 1)]
            parts.append(x[sel, :b, :].reshape(CS * b, DIM))
        m = dict(tabs)
        m["x"] = np.ascontiguousarray(np.concatenate(parts, axis=0))
        m["n_i"] = np.ascontiguousarray(n[cs])
        in_maps.append(m)
    return nc, in_maps, cores_sets


def kernel(**inputs):
    nc, in_maps, cores_sets = prepare(inputs)
    res = bass_utils.run_bass_kernel_spmd(nc, in_maps, list(range(NCORES)))
    z = np.empty((B, HID), np.float32)
    for c in range(NCORES):
        z[cores_sets[c]] = res.results[c]["z_out"]
    return z
